# revision 3
# baseline (speedup 1.0000x reference)
"""CharCNN embedding kernel for 8 Trainium2 NeuronCores (pure data parallel).

Math: CHAR_VOCAB == 128 == PE partitions, so char-embedding + Conv1d collapse
into one-hot matmuls with fused tables Phi_j = char_emb @ w_k[:, :, j].T.
conv_out[:, l] = sum_j Phi_j[char[l+j], :].

v2 redesign vs the bf16 baseline (172.8 us):
  * Taps run as fp8e4 (e4m3) DoubleRow matmuls: each instruction contracts
    TWO 128-deep k-tiles per output column at 0.5 cyc/col.  The two tiles
    are (phi_hi_j, phi_lo_j), an error-feedback split of Phi_j into fp8
    value + fp8 residual, so tap accuracy BEATS bf16 while the PE runs 4x
    faster than the bf16 baseline taps (5 pair-matmuls per 512-col chunk).
  * The char broadcast leaves the PE entirely: gathered uint8 chars bounce
    through DRAM and 16 grouped DMAs broadcast each chunk's 516-byte row
    (overlapping-window AP supplies the 4-col halo) to all 128 partitions.
  * One-hot via DVE is_equal on uint8 chars vs an fp32 iota column, all
    operands SBUF -> 2x_2p mode (0.5 cyc/elem).  Row 0 is never a real
    char (char_table values are 1..127), so it is pre-set ONCE with 1s at
    word-start columns; phi rows 0 of taps 2/4 carry -224 so invalid conv
    tail positions (l>=28 branch5, l>=30 branch3) mask themselves inside
    the accumulating matmuls: zero per-block DVE mask ops.
  * PSUM evacuation with fused ReLU+bias on the otherwise-idle Activation
    engine (reference applies ReLU before the length-max; monotonicity
    makes relu-then-max == max-then-relu, and masked -224 slots land at
    relu's 0 floor exactly like the reference's all-negative case).
  * The 32-position max splits by 4-block group: 'D' groups tensor_reduce
    PSUM directly on DVE (ReLU+bias applied later on ACT), 'P' groups
    tree-max on GpSimd from the ACT-evacuated bf16 copy (GpSimd has no
    PSUM port) with a short DVE finish.
  * lin_b rides the linear matmul as contraction row 120 against a
    constant-1 feats row; ACT copies the PSUM result to the DMA stage.

Walrus accepts ONE semaphore wait per engine instruction; every steady-
state instruction depends on at most one foreign engine.  PE ldweights /
ACT tiny-copy / Pool engine_nop dep-carriers absorb second-engine ticks
(baseline's pattern), and all DMA deps ride the single SP HWDGE queue
semaphore.
"""

import os
import sys

for _p in ("/root/.axon_site", "/root/.axon_site/_ro/trn_rl_repo",
           "/root/.axon_site/_ro/pypackages", "/opt/trn_rl_repo"):
    if os.path.isdir(_p) and _p not in sys.path:
        sys.path.append(_p)

import numpy as np
import ml_dtypes

import bass_rust
import concourse.bass as bass
import concourse.mybir as mybir
import concourse.tile as tile
from concourse.tile import add_dep_helper
from concourse.tile_scheduler import N_PROCS
from concourse.vector_clock import ScopedClock, VectorClock
from concourse.bass_utils import run_bass_kernel_spmd

dt = mybir.dt
AF = mybir.ActivationFunctionType
ALU = mybir.AluOpType
PM = mybir.MatmulPerfMode

N_CORES = 8
B, S = 64, 256
W = (B * S) // N_CORES          # words per core: 2048
L = 32                          # max word length
V = 128                         # char vocab
F_TOT = 120                     # 30 + 40 + 50 filters
EMB = 50                        # output embed size
VOCAB = 50000

WPC = 16                        # words per chunk
CW = WPC * L                    # 512 chunk columns
SEG = CW + 4                    # chunk cols incl 4-col halo
N_CHUNK = W // WPC              # 128
N_BLOCK = N_CHUNK // 2          # 64 (2 chunks = 32 words = 2 PSUM banks)
WPB = 2 * WPC                   # words per block: 32
N_GATHER = 16                   # gather i covers words 128i+p
GB = 8                          # chunks per broadcast DMA group
N_GROUP = N_CHUNK // GB         # 16 broadcast groups
N_PAIR = 5                      # DoubleRow pair-matmuls per chunk
# phi tables are stored scaled by PHI_SCALE so the fp8 hi+lo error-feedback
# residuals land in e4m3's NORMAL range (unscaled residuals ~1e-3 hit the
# 2^-9 subnormal quantum, capping table accuracy at ~0.5%; scaled, ~0.05%).
# The 1/PHI_SCALE rides the ACT featsr/evac `scale` for free.
PHI_SCALE = 64.0
MASK_VAL = -224.0               # exact in e4m3; overwhelms |64*garbage| < ~96

F8 = dt.float8e4                # e4m3
NP_F8 = ml_dtypes.float8_e4m3
ONE_F8 = np.float32(1.0).astype(NP_F8).view(np.uint8).item()

# reducer kind per 4-block group: 'D' = DVE direct, 'P' = ACT+Pool tree.
# Leading D groups let Pool drain the startup gathers first.
PATTERN = "D" * 16
# one-hot producer per chunk: ~55% on ACT (2-pass |c-v| trick) to relieve
# DVE, which carries the whole PSUM max-reduce (Pool has no walrus-legal
# compute and DMA-max is rejected, so DVE+ACT are the only scan engines).
# Fraction tuned by timeline-sim sweep; evenly interleaved placement.
def _oh_pattern(frac=0.55):
    acc, sel = 0.0, []
    for _ in range(N_CHUNK):
        acc += frac
        if acc >= 1.0:
            acc -= 1.0
            sel.append(True)
        else:
            sel.append(False)
    return sel


OH_ACT = _oh_pattern()
assert len(PATTERN) == N_BLOCK // 4
FIN_LAG = 3                     # iterations between P-block taps and DVE finish

_PROGRAM_CACHE = {}


class OneWaitTileContext(tile.TileContext):
    """TileContext whose teardown drain obeys walrus's one-semaphore-wait-
    per-instruction limit (waits split across per-proc nops)."""

    def _drain_and_barrier(self, tick_clock, wait_clock):
        gc = tick_clock.global_clock
        for p in range(N_PROCS):
            tick = gc.peek_next(p) - 1
            if tick <= 0:
                continue
            vec = [0] * N_PROCS
            vec[p] = tick
            nop = self.nc.sync.nop(nofuse=True)
            wait_clock.add_sem_waits(
                nop.ins, ScopedClock({None: VectorClock(vec)})
            )
        self.nc.sync.drain()
        self.nc.all_engine_barrier()
        assert self.sems is not None
        popped = self.nc._tile_sem_poison_stack.pop()
        assert popped is self._sem_poison
        self.nc.clear_and_free_semaphores(list(self.sems.allocated().values()))
        self.nc.all_engine_barrier()


def _split_excess_waits(nc):
    """Walrus codegen accepts at most ONE semaphore wait per instruction.
    The tile framework can emit more (e.g. a DMA whose HWDGE lane semaphore
    is being reused carries a lane-FIFO wait on top of its data wait).
    Hoist all but the last wait onto nofuse NoOps on the same engine,
    inserted immediately before the offender — the sequencer executes them
    first, preserving ordering exactly (this mirrors Bacc's
    generate_event_semaphores pass, which the walrus path lacks)."""
    fn = nc.m.functions[0]
    # The framework emits tiny const-scalar Memsets on Pool; walrus's
    # TRN2 codegen rejects Memset on the Pool engine — move them to DVE
    # (they run once in the preamble with no waits).
    for blk in fn.blocks:
        for ins in blk.instructions:
            if ins.opcode == "Memset" and ins.engine == mybir.EngineType.Pool:
                ins.engine = mybir.EngineType.DVE
    n_split = 0
    for blk in fn.blocks:
        insts = blk.instructions
        idx = 0
        while idx < len(insts):
            ins = insts[idx]
            si = ins.sync_info
            if (
                si is not None
                and si.on_wait is not None
                and len(si.on_wait) > 1
                and ins.opcode != "EventSemaphore"
            ):
                waits = list(si.on_wait)
                for w in waits[:-1]:
                    nop = mybir.InstNoOp(
                        name=f"{ins.name}-wsplit{n_split}",
                        sync_info=mybir.SyncInfo(on_wait=[w], on_update=[]),
                        bass_nofuse=True,
                        engine=ins.engine,
                    )
                    insts.insert(idx, nop)
                    idx += 1
                    n_split += 1
                si.on_wait = [waits[-1]]
            idx += 1
    return n_split


def _build_program():
    if "nc" in _PROGRAM_CACHE:
        return _PROGRAM_CACHE["nc"]

    nc = bass.Bass()
    widx_d = nc.dram_tensor("widx", (128, N_GATHER), dt.int32, kind="ExternalInput")
    ctab_d = nc.dram_tensor("ctab8", (VOCAB, L), dt.uint8, kind="ExternalInput")
    phip_d = nc.dram_tensor("phipack", (V, N_PAIR * 2 * V), F8, kind="ExternalInput")
    vpack_d = nc.dram_tensor("vpack", (128, 3), dt.float32, kind="ExternalInput")
    linw_d = nc.dram_tensor("linw", (F_TOT + 1, EMB), dt.bfloat16, kind="ExternalInput")
    row0_d = nc.dram_tensor("row0pat", (1, N_CHUNK * SEG), dt.uint8, kind="ExternalInput")
    ones_d = nc.dram_tensor("onesrow", (1, N_BLOCK * L), dt.bfloat16, kind="ExternalInput")
    # +WPC+4 slack rows: group 15's overlapping window reads 4 bytes past
    # word 2047; the garbage self-masks (halo cols feed only masked taps)
    stage_d = nc.dram_tensor("chstage", (W + WPC + 4, L), dt.uint8, kind="Internal")
    out_d = nc.dram_tensor("out", (W, EMB), dt.float32, kind="ExternalOutput")

    with OneWaitTileContext(nc) as tc:
        with (
            tc.tile_pool(name="consts", bufs=1) as consts,
            tc.tile_pool(name="relup", bufs=3) as relup,
            tc.tile_pool(name="treep", bufs=3) as treep,
            tc.tile_pool(name="psm", bufs=3, space="PSUM") as psm,
            tc.tile_pool(name="psl", bufs=2, space="PSUM") as psl,
        ):
            # ---- constants (all on the SP HWDGE queue: one DMA semaphore) ----
            feats = consts.tile((F_TOT + 1, N_BLOCK * L), dt.bfloat16)
            nc.sync.dma_start(feats[F_TOT : F_TOT + 1, :], ones_d[:])
            oh_all = consts.tile((128, N_CHUNK * SEG), F8)
            bc_all = consts.tile((128, N_CHUNK * SEG), dt.uint8)
            # bc row 0 is a one-time pattern: byte 0 at word-start columns
            # (0 == iota[0] -> is_equal emits the mask 1s there), 255 else.
            # Broadcasts below write rows 1:128 only, so each chunk's
            # is_equal regenerates mask row 0 of the one-hot for free.
            nc.sync.dma_start(bc_all[0:1, :], row0_d[:])
            vpack_sb = consts.tile((128, 3), dt.float32)
            nc.sync.dma_start(vpack_sb[:], vpack_d[:])
            linw_sb = consts.tile((F_TOT + 1, EMB), dt.bfloat16)
            nc.sync.dma_start(linw_sb[:], linw_d[:])
            widx_sb = consts.tile((128, N_GATHER), dt.int32)
            nc.sync.dma_start(widx_sb[:], widx_d[:])
            phip_sb = consts.tile((V, N_PAIR * 2 * V), F8)
            nc.sync.dma_start(phip_sb[:], phip_d[:])

            iota_sb = vpack_sb[:, 0:1]
            bias_sb = vpack_sb[:, 1:2]
            negiota_sb = vpack_sb[:, 2:3]

            chars_sb = consts.tile((128, N_GATHER * L), dt.uint8)
            draft_all = consts.tile((F_TOT, N_BLOCK * L), dt.float32)
            out_stage = consts.tile((128, (W // 128) * EMB), dt.float32)
            scratch1 = consts.tile((1, 4), dt.float32)

            # ---- one-time pre-touches absorbing const-DMA lane ticks ----
            # (each DMA has its own HWDGE lane semaphore; an engine that
            # observes a lane's tick once never re-waits on earlier ticks)
            nc.tensor.ldweights(weights=phip_sb[:, 0:1])
            nc.tensor.ldweights(weights=linw_sb[:, 0:1])
            nc.tensor.ldweights(weights=feats[:, 0:1])
            nc.scalar.activation(
                out=scratch1[0:1, 0:1], in_=vpack_sb[0:1, 1:2], func=AF.Copy
            )
            nc.scalar.activation(
                out=scratch1[0:1, 1:2],
                in_=bc_all[0:1, 0:1], func=AF.Copy
            )
            dve_scr = consts.tile((1, 4), dt.float32)
            nc.vector.tensor_copy(dve_scr[0:1, 0:1], bc_all[0:1, 0:1])
            nc.vector.tensor_copy(dve_scr[0:1, 1:2], vpack_sb[0:1, 0:1])
            pool_scratch_reg = nc.gpsimd.alloc_register("poolcar")

            # ---- gather -> stage -> broadcast, software-pipelined ----
            gathers = []
            for i in range(N_GATHER):
                g = nc.gpsimd.indirect_dma_start(
                    out=chars_sb[:, i * L : (i + 1) * L],
                    out_offset=None,
                    in_=ctab_d[:],
                    in_offset=bass.IndirectOffsetOnAxis(
                        ap=widx_sb[:, i : i + 1], axis=0),
                )
                gathers.append(g)

            stages = [None] * N_GATHER

            def emit_stage(i):
                s = nc.sync.dma_start(
                    stage_d[128 * i : 128 * (i + 1), :],
                    chars_sb[:, i * L : (i + 1) * L],
                )
                add_dep_helper(s.ins, gathers[i].ins, reason="stage after gather")
                stages[i] = s

            def emit_bcast(g):
                in_ap = bass_rust.AP(
                    tensor=stage_d[:].tensor,
                    ap=[[0, 127], [CW, GB], [1, SEG]],
                    offset=g * GB * CW,
                )
                bcast = nc.sync.dma_start(
                    bc_all[1:128, g * GB * SEG : (g + 1) * GB * SEG], in_ap
                )
                add_dep_helper(bcast.ins, stages[g].ins, reason="bcast after stage")
                if g + 1 < N_GATHER and stages[g + 1] is not None:
                    add_dep_helper(bcast.ins, stages[g + 1].ins, reason="halo")

            emit_stage(0)
            emit_stage(1)
            emit_bcast(0)
            for i in range(2, N_GATHER):
                emit_stage(i)
                emit_bcast(i - 1)
            emit_bcast(N_GROUP - 1)

            # ---- steady-state pipeline ----
            pm_tiles = {}
            pm_freer = {}          # block -> ins whose completion frees pm slot
            onehots = {}
            # P-path bookkeeping indexed by P-allocation order (pool cycling)
            p_l1 = []              # Pool L1 ins per P-block (relu-tile readers)
            p_fin = []             # DVE finish ins per P-block (tree readers)
            p_tree_jobs = {}       # block -> (tree_t3_ap, l3_ins)
            prev_psl_copy = [None, None]
            group_featsrc = {}

            abs_pool = relup  # reuse pool; (128, SEG) bf16 scratch tiles

            def emit_onehot(c):
                seg = slice(c * SEG, (c + 1) * SEG)
                if OH_ACT[c]:
                    # ACT 2-pass: a = |c - v|; oh = relu(1 - a). Integer
                    # chars make both passes exact: a=0 iff match.
                    a = abs_pool.tile((128, SEG), dt.bfloat16, tag="abs")
                    nc.scalar.activation(
                        out=a[:], in_=bc_all[:, seg], func=AF.Abs,
                        bias=negiota_sb[:], scale=1.0,
                    )
                    onehots[c] = nc.scalar.activation(
                        out=oh_all[:, seg], in_=a[:], func=AF.Relu,
                        bias=1.0, scale=-1.0,
                    )
                else:
                    onehots[c] = nc.vector.tensor_scalar(
                        out=oh_all[:, seg], in0=bc_all[:, seg],
                        scalar1=iota_sb[:], scalar2=None, op0=ALU.is_equal,
                    )

            def emit_taps(b):
                pm = psm.tile((128, 2 * CW), dt.float32, tag="pm")
                pm_tiles[b] = pm
                freer = pm_freer.pop(b - 3, None)
                if freer is not None:
                    ldw = nc.tensor.ldweights(weights=phip_sb[:, 1:2])
                    add_dep_helper(ldw.ins, freer.ins, reason="pm WAR absorb")
                for h in range(2):
                    c = 2 * b + h
                    oh = onehots.pop(c)
                    base = c * SEG
                    for j in range(N_PAIR):
                        lhsT = phip_sb[:, j * 2 * V : (j + 1) * 2 * V].rearrange(
                            "p (two m) -> p two m", two=2
                        )
                        rhs = (
                            oh_all[:, base + j : base + j + CW]
                            .unsqueeze(1)
                            .broadcast_to((128, 2, CW))
                        )
                        mm = nc.tensor.matmul(
                            pm[:, h * CW : (h + 1) * CW],
                            lhsT=lhsT,
                            rhs=rhs,
                            start=(j == 0),
                            stop=(j == N_PAIR - 1),
                            perf_mode=PM.DoubleRow,
                        )
                        if h == 0 and j == 0:
                            add_dep_helper(mm.ins, oh.ins, reason="onehot ready")

            def emit_reduce_D(b):
                pm = pm_tiles.pop(b)
                pm3 = pm[0:F_TOT, :].rearrange("p (w l) -> p w l", l=L)
                r = nc.vector.tensor_reduce(
                    out=draft_all[:, b * L : (b + 1) * L],
                    in_=pm3, axis=mybir.AxisListType.X, op=ALU.max,
                )
                pm_freer[b] = r

            def emit_reduce_P(b):
                pm = pm_tiles.pop(b)
                k = len(p_l1)
                # ACT: relu(y + bias) -> bf16 SBUF; carrier absorbs the Pool
                # tick that frees this relu-pool slot (L1 three P-blocks ago)
                relu = relup.tile((F_TOT, WPB * L), dt.bfloat16, tag="relu")
                if k >= 3:
                    acar = nc.scalar.activation(
                        out=scratch1[0:1, 1:2], in_=scratch1[0:1, 0:1],
                        func=AF.Copy,
                    )
                    add_dep_helper(acar.ins, p_l1[k - 3].ins, reason="relu WAR")
                ev = nc.scalar.activation(
                    out=relu[:], in_=pm[0:F_TOT, :], func=AF.Relu,
                    bias=bias_sb[0:F_TOT, :], scale=1.0 / PHI_SCALE,
                )
                pm_freer[b] = ev
                r3 = relu[:].rearrange("p (w l) -> p w l", l=L)
                t = treep.tile((F_TOT, WPB * 28), dt.bfloat16, tag="tree")
                t1 = t[:, 0 : WPB * 16].rearrange("p (w l) -> p w l", l=16)
                t2 = t[:, WPB * 16 : WPB * 24].rearrange("p (w l) -> p w l", l=8)
                t3 = t[:, WPB * 24 : WPB * 28].rearrange("p (w l) -> p w l", l=4)
                # Pool carrier: tree-pool slot WAR on the DVE finish 3 ago
                # (reg_mov: RegisterMove is walrus-legal on Pool, InstISA
                # engine_nop is not)
                if k >= 3:
                    ncar = nc.gpsimd.reg_mov(pool_scratch_reg, 0)
                    add_dep_helper(ncar.ins, p_fin[k - 3].ins, reason="tree WAR")
                l1 = nc.gpsimd.tensor_tensor(
                    out=t1, in0=r3[:, :, 0:16], in1=r3[:, :, 16:32], op=ALU.max)
                add_dep_helper(l1.ins, ev.ins, reason="tree after evac")
                l2 = nc.gpsimd.tensor_tensor(
                    out=t2, in0=t1[:, :, 0:8], in1=t1[:, :, 8:16], op=ALU.max)
                l3 = nc.gpsimd.tensor_tensor(
                    out=t3, in0=t2[:, :, 0:4], in1=t2[:, :, 4:8], op=ALU.max)
                p_l1.append(l1)
                p_tree_jobs[b] = (t3, l3)

            def emit_fin_P(b):
                t3, l3 = p_tree_jobs.pop(b)
                fin = nc.vector.tensor_reduce(
                    out=feats[0:F_TOT, b * L : (b + 1) * L],
                    in_=t3, axis=mybir.AxisListType.X, op=ALU.max,
                )
                add_dep_helper(fin.ins, l3.ins, reason="finish after tree")
                p_fin.append(fin)
                group_featsrc[b // 4] = fin

            def emit_linear(g):
                gs = slice(g * 4 * L, (g + 1) * 4 * L)
                if PATTERN[g] == "D":
                    # featsr = relu(draft/PHI_SCALE + bias), to bf16, on ACT
                    fr = nc.scalar.activation(
                        out=feats[0:F_TOT, gs], in_=draft_all[:, gs],
                        func=AF.Relu, bias=bias_sb[0:F_TOT, :],
                        scale=1.0 / PHI_SCALE,
                    )
                    group_featsrc[g] = fr
                pl = psl.tile((128, EMB), dt.float32, tag="pl")
                # PE carrier: psl slot WAR on the ACT copy two groups ago
                old = prev_psl_copy.pop(0)
                if old is not None:
                    ldw = nc.tensor.ldweights(weights=phip_sb[:, 2:3])
                    add_dep_helper(ldw.ins, old.ins, reason="psl WAR absorb")
                lm = nc.tensor.matmul(
                    pl[:],
                    lhsT=feats[:, gs],
                    rhs=linw_sb[:],
                    start=True,
                    stop=True,
                )
                add_dep_helper(lm.ins, group_featsrc[g].ins, reason="feats ready")
                cp = nc.scalar.activation(
                    out=out_stage[:, g * EMB : (g + 1) * EMB], in_=pl[:],
                    func=AF.Copy,
                )
                prev_psl_copy.append(cp)

            for it in range(N_BLOCK + FIN_LAG + 2):
                if it < N_BLOCK:
                    emit_onehot(2 * it)
                    emit_onehot(2 * it + 1)
                b = it - 1
                if 0 <= b < N_BLOCK:
                    emit_taps(b)
                    if PATTERN[b // 4] == "D":
                        emit_reduce_D(b)
                    else:
                        emit_reduce_P(b)
                bf = it - FIN_LAG
                if 0 <= bf < N_BLOCK and PATTERN[bf // 4] == "P":
                    emit_fin_P(bf)
                bl = it - FIN_LAG - 1
                if bl >= 3 and (bl + 1) % 4 == 0:
                    emit_linear(bl // 4)

            nc.sync.dma_start(
                out_d[:].rearrange("(g p) e -> p g e", p=128),
                out_stage[:].rearrange("p (g e) -> p g e", e=EMB),
            )

    _split_excess_waits(nc)
    _PROGRAM_CACHE["nc"] = nc
    return nc


def _host_prep(inputs):
    word_idxs = np.asarray(inputs["word_idxs"])
    char_table = np.asarray(inputs["char_table"], dtype=np.int64)
    char_emb = np.asarray(inputs["char_emb"], dtype=np.float32)
    w1 = np.asarray(inputs["w1"], dtype=np.float32)
    w3 = np.asarray(inputs["w3"], dtype=np.float32)
    w5 = np.asarray(inputs["w5"], dtype=np.float32)
    lin_w = np.asarray(inputs["lin_w"], dtype=np.float32)
    lin_b = np.asarray(inputs["lin_b"], dtype=np.float32)
    b1 = np.asarray(inputs["b1"], dtype=np.float32)
    b3 = np.asarray(inputs["b3"], dtype=np.float32)
    b5 = np.asarray(inputs["b5"], dtype=np.float32)

    ctab8 = np.ascontiguousarray(char_table.astype(np.uint8))

    phi = np.zeros((5, V, V), dtype=np.float32)
    phi[0, :, 0:30] = char_emb @ w1[:, :, 0].T
    for j in range(3):
        phi[j, :, 30:70] = char_emb @ w3[:, :, j].T
    for j in range(5):
        phi[j, :, 70:120] = char_emb @ w5[:, :, j].T
    phi *= PHI_SCALE
    # invalid-tail masks ride one-hot row 0 (char 0 never occurs):
    # tap2 covers l in {30,31} for branch3+5, tap4 covers {28,29} for branch5
    # (mask stays raw -224: vs |scaled garbage| < ~96 it still dominates)
    phi[2, 0, 30:120] = MASK_VAL
    phi[4, 0, 70:120] = MASK_VAL

    phi_hi = phi.astype(NP_F8)
    phi_lo = (phi - phi_hi.astype(np.float32)).astype(NP_F8)
    phipack = np.zeros((V, N_PAIR * 2 * V), dtype=NP_F8)
    for j in range(5):
        phipack[:, j * 2 * V : j * 2 * V + V] = phi_hi[j]
        phipack[:, j * 2 * V + V : (j + 1) * 2 * V] = phi_lo[j]

    vpack = np.zeros((128, 3), dtype=np.float32)
    vpack[:, 0] = np.arange(V, dtype=np.float32)
    vpack[0:F_TOT, 1] = np.concatenate([b1, b3, b5])
    vpack[:, 2] = -np.arange(V, dtype=np.float32)

    linw = np.zeros((F_TOT + 1, EMB), dtype=ml_dtypes.bfloat16)
    linw[0:F_TOT] = lin_w.T.astype(ml_dtypes.bfloat16)
    linw[F_TOT] = lin_b.astype(ml_dtypes.bfloat16)

    # bc row 0 pattern: 0 at word-start columns (matches iota[0]=0 so
    # is_equal emits 1s there), 255 (never a char, never matches) elsewhere
    row0 = np.full((1, N_CHUNK * SEG), 255, dtype=np.uint8)
    for seg in range(N_CHUNK):
        for m in range(WPC + 1):
            for k in (0, 1):
                p = m * L + k
                if p < SEG:
                    row0[0, seg * SEG + p] = 0

    ones = np.ones((1, N_BLOCK * L), dtype=ml_dtypes.bfloat16)

    flat = np.ascontiguousarray(word_idxs, dtype=np.int32).reshape(-1)
    common = {
        "ctab8": ctab8,
        "phipack": phipack,
        "vpack": vpack,
        "linw": linw,
        "row0pat": row0,
        "onesrow": ones,
    }
    in_maps = []
    for i in range(N_CORES):
        shard = flat[i * W : (i + 1) * W].reshape(N_GATHER, 128).T
        in_maps.append({"widx": np.ascontiguousarray(shard), **common})
    return in_maps


def run(inputs, trace=False, **kw):
    nc = _build_program()
    in_maps = _host_prep(inputs)
    res = run_bass_kernel_spmd(
        nc, in_maps, core_ids=list(range(N_CORES)), trace=trace, **kw
    )
    out = np.concatenate([r["out"] for r in res.results], axis=0)
    return out.reshape(B, S, EMB).astype(np.float32), res


def kernel(**inputs):
    out, _ = run(inputs, trace=False)
    return out


# revision 4
# speedup vs baseline: 3.6373x; 3.6373x over previous
"""CharCNN embedding kernel for 8 Trainium2 NeuronCores (pure data parallel).

Math: CHAR_VOCAB == 128 == PE partitions, so char-embedding + Conv1d collapse
into one-hot matmuls with fused tables Phi_j = char_emb @ w_k[:, :, j].T.
conv_out[:, l] = sum_j Phi_j[char[l+j], :].

v2 redesign vs the bf16 baseline (172.8 us measured; timeline-sim 243 us
-> 122 us for this version):
  * Taps run as fp8e4 (e4m3) DoubleRow matmuls: each instruction contracts
    TWO 128-deep k-tiles per output column at 0.5 cyc/col.  The two tiles
    are (phi_hi_j, phi_lo_j), an error-feedback split of PHI_SCALE*Phi_j
    into fp8 value + fp8 residual (the scale keeps residuals out of
    e4m3's 2^-9 subnormal quantum), so tap accuracy BEATS bf16 while the
    PE runs 4x faster than the bf16 baseline taps (5 pair-matmuls per
    512-col chunk; the 1/PHI_SCALE rides the ACT featsr `scale`).
  * The char broadcast leaves the PE entirely: gathered uint8 chars bounce
    through DRAM and 16 grouped DMAs broadcast each chunk's 516-byte row
    (overlapping-window AP supplies the 4-col halo) to all 128 partitions;
    one InstDMACopy's descriptors spread across all 16 SDMA engines.
  * One-hot from the broadcast bytes, split ~45/55 between DVE (is_equal
    vs an fp32 iota column; all-SBUF operands hit the 2x_2p perf mode)
    and the otherwise-idle ACT engine (2 exact integer passes:
    a = Abs(c - v), oh = Relu(1 - a)).  GpSimd cannot help: walrus
    rejects every Pool compute opcode (Bacc-only Q7 dispatch) and
    DMACopy accum only supports bypass/add, so DVE+ACT are the only
    scan engines and DVE keeps the whole PSUM max-reduce.
  * Row 0 of the one-hot is never a real char (char_table values are
    1..127): the broadcast writes partitions 1:128 only and a one-time
    pattern in bc row 0 (byte 0 == iota[0] at word-start columns) makes
    each chunk's one-hot op regenerate mask 1s there; phi rows 0 of taps
    2/4 carry -224 so the invalid conv tail positions (l>=28 branch5,
    l>=30 branch3) mask themselves inside the accumulating matmuls:
    zero per-block mask instructions.
  * DVE tensor_reduce maxes each block's PSUM directly; ACT applies
    relu(max/PHI_SCALE + bias) per 4-block group into the bf16 linear
    lhsT (monotonicity makes relu-then-max == max-then-relu, and masked
    -224 slots land at relu's 0 floor exactly like the reference).
  * lin_b rides the linear matmul as contraction row 120 against a
    constant-1 feats row; ACT copies the PSUM result to the DMA stage.

Walrus accepts ONE semaphore wait per instruction; steady-state
instructions depend on at most one foreign engine (PE ldweights carriers
absorb pm-slot WAR ticks, one-time pre-touches absorb const-DMA lane
ticks), and _split_excess_waits legalizes whatever remains — notably
HWDGE-lane-reuse FIFO waits colliding with data waits on DMAs.
"""

import os
import sys

for _p in ("/root/.axon_site", "/root/.axon_site/_ro/trn_rl_repo",
           "/root/.axon_site/_ro/pypackages", "/opt/trn_rl_repo"):
    if os.path.isdir(_p) and _p not in sys.path:
        sys.path.append(_p)

import numpy as np
import ml_dtypes

import bass_rust
import concourse.bass as bass
import concourse.mybir as mybir
import concourse.tile as tile
from concourse.tile import add_dep_helper
from concourse.tile_scheduler import N_PROCS
from concourse.vector_clock import ScopedClock, VectorClock
from concourse.bass_utils import run_bass_kernel_spmd

dt = mybir.dt
AF = mybir.ActivationFunctionType
ALU = mybir.AluOpType
PM = mybir.MatmulPerfMode

N_CORES = 8
B, S = 64, 256
W = (B * S) // N_CORES          # words per core: 2048
L = 32                          # max word length
V = 128                         # char vocab
F_TOT = 120                     # 30 + 40 + 50 filters
EMB = 50                        # output embed size
VOCAB = 50000

WPC = 16                        # words per chunk
CW = WPC * L                    # 512 chunk columns
SEG = CW + 4                    # chunk cols incl 4-col halo
N_CHUNK = W // WPC              # 128
N_BLOCK = N_CHUNK // 2          # 64 (2 chunks = 32 words = 2 PSUM banks)
WPB = 2 * WPC                   # words per block: 32
N_GATHER = 16                   # gather i covers words 128i+p
GB = 8                          # chunks per broadcast DMA group
N_GROUP = N_CHUNK // GB         # 16 broadcast groups
N_PAIR = 5                      # DoubleRow pair-matmuls per chunk
# phi tables are stored scaled by PHI_SCALE so the fp8 hi+lo error-feedback
# residuals land in e4m3's NORMAL range (unscaled residuals ~1e-3 hit the
# 2^-9 subnormal quantum, capping table accuracy at ~0.5%; scaled, ~0.05%).
# The 1/PHI_SCALE rides the ACT featsr/evac `scale` for free.
PHI_SCALE = 64.0
MASK_VAL = -224.0               # exact in e4m3; overwhelms |64*garbage| < ~96

F8 = dt.float8e4                # e4m3
NP_F8 = ml_dtypes.float8_e4m3
ONE_F8 = np.float32(1.0).astype(NP_F8).view(np.uint8).item()

# reducer kind per 4-block group: 'D' = DVE direct, 'P' = ACT+Pool tree.
# Leading D groups let Pool drain the startup gathers first.
PATTERN = "D" * 16
# one-hot producer per chunk: ~55% on ACT (2-pass |c-v| trick) to relieve
# DVE, which carries the whole PSUM max-reduce (Pool has no walrus-legal
# compute and DMA-max is rejected, so DVE+ACT are the only scan engines).
# Fraction tuned by timeline-sim sweep; evenly interleaved placement.
def _oh_pattern(frac=0.55):
    acc, sel = 0.0, []
    for _ in range(N_CHUNK):
        acc += frac
        if acc >= 1.0:
            acc -= 1.0
            sel.append(True)
        else:
            sel.append(False)
    return sel


OH_ACT = _oh_pattern()
assert len(PATTERN) == N_BLOCK // 4
FIN_LAG = 3                     # iterations between P-block taps and DVE finish

_PROGRAM_CACHE = {}


class OneWaitTileContext(tile.TileContext):
    """TileContext whose teardown drain obeys walrus's one-semaphore-wait-
    per-instruction limit (waits split across per-proc nops)."""

    def _drain_and_barrier(self, tick_clock, wait_clock):
        gc = tick_clock.global_clock
        for p in range(N_PROCS):
            tick = gc.peek_next(p) - 1
            if tick <= 0:
                continue
            vec = [0] * N_PROCS
            vec[p] = tick
            nop = self.nc.sync.nop(nofuse=True)
            wait_clock.add_sem_waits(
                nop.ins, ScopedClock({None: VectorClock(vec)})
            )
        self.nc.sync.drain()
        self.nc.all_engine_barrier()
        assert self.sems is not None
        popped = self.nc._tile_sem_poison_stack.pop()
        assert popped is self._sem_poison
        self.nc.clear_and_free_semaphores(list(self.sems.allocated().values()))
        self.nc.all_engine_barrier()


def _split_excess_waits(nc):
    """Walrus codegen accepts at most ONE semaphore wait per instruction.
    The tile framework can emit more (e.g. a DMA whose HWDGE lane semaphore
    is being reused carries a lane-FIFO wait on top of its data wait).
    Hoist all but the last wait onto nofuse NoOps on the same engine,
    inserted immediately before the offender — the sequencer executes them
    first, preserving ordering exactly (this mirrors Bacc's
    generate_event_semaphores pass, which the walrus path lacks)."""
    fn = nc.m.functions[0]
    # The framework emits tiny const-scalar Memsets on Pool; walrus's
    # TRN2 codegen rejects Memset on the Pool engine — move them to DVE
    # (they run once in the preamble with no waits).
    for blk in fn.blocks:
        for ins in blk.instructions:
            if ins.opcode == "Memset" and ins.engine == mybir.EngineType.Pool:
                ins.engine = mybir.EngineType.DVE
    n_split = 0
    for blk in fn.blocks:
        insts = blk.instructions
        idx = 0
        while idx < len(insts):
            ins = insts[idx]
            si = ins.sync_info
            if (
                si is not None
                and si.on_wait is not None
                and len(si.on_wait) > 1
                and ins.opcode != "EventSemaphore"
            ):
                waits = list(si.on_wait)
                for w in waits[:-1]:
                    nop = mybir.InstNoOp(
                        name=f"{ins.name}-wsplit{n_split}",
                        sync_info=mybir.SyncInfo(on_wait=[w], on_update=[]),
                        bass_nofuse=True,
                        engine=ins.engine,
                    )
                    insts.insert(idx, nop)
                    idx += 1
                    n_split += 1
                si.on_wait = [waits[-1]]
            idx += 1
    return n_split


def _build_program():
    if "nc" in _PROGRAM_CACHE:
        return _PROGRAM_CACHE["nc"]

    nc = bass.Bass()
    widx_d = nc.dram_tensor("widx", (128, N_GATHER), dt.int32, kind="ExternalInput")
    ctab_d = nc.dram_tensor("ctab8", (VOCAB, L), dt.uint8, kind="ExternalInput")
    phip_d = nc.dram_tensor("phipack", (V, N_PAIR * 2 * V), F8, kind="ExternalInput")
    vpack_d = nc.dram_tensor("vpack", (128, 3), dt.float32, kind="ExternalInput")
    linw_d = nc.dram_tensor("linw", (F_TOT + 1, EMB), dt.bfloat16, kind="ExternalInput")
    row0_d = nc.dram_tensor("row0pat", (1, N_CHUNK * SEG), dt.uint8, kind="ExternalInput")
    ones_d = nc.dram_tensor("onesrow", (1, N_BLOCK * L), dt.bfloat16, kind="ExternalInput")
    # +WPC+4 slack rows: group 15's overlapping window reads 4 bytes past
    # word 2047; the garbage self-masks (halo cols feed only masked taps)
    stage_d = nc.dram_tensor("chstage", (W + WPC + 4, L), dt.uint8, kind="Internal")
    out_d = nc.dram_tensor("out", (W, EMB), dt.float32, kind="ExternalOutput")

    with OneWaitTileContext(nc) as tc:
        with (
            tc.tile_pool(name="consts", bufs=1) as consts,
            tc.tile_pool(name="relup", bufs=3) as relup,
            tc.tile_pool(name="treep", bufs=3) as treep,
            tc.tile_pool(name="psm", bufs=3, space="PSUM") as psm,
            tc.tile_pool(name="psl", bufs=2, space="PSUM") as psl,
        ):
            # ---- constants (all on the SP HWDGE queue: one DMA semaphore) ----
            feats = consts.tile((F_TOT + 1, N_BLOCK * L), dt.bfloat16)
            nc.sync.dma_start(feats[F_TOT : F_TOT + 1, :], ones_d[:])
            oh_all = consts.tile((128, N_CHUNK * SEG), F8)
            bc_all = consts.tile((128, N_CHUNK * SEG), dt.uint8)
            # bc row 0 is a one-time pattern: byte 0 at word-start columns
            # (0 == iota[0] -> is_equal emits the mask 1s there), 255 else.
            # Broadcasts below write rows 1:128 only, so each chunk's
            # is_equal regenerates mask row 0 of the one-hot for free.
            nc.sync.dma_start(bc_all[0:1, :], row0_d[:])
            vpack_sb = consts.tile((128, 3), dt.float32)
            nc.sync.dma_start(vpack_sb[:], vpack_d[:])
            linw_sb = consts.tile((F_TOT + 1, EMB), dt.bfloat16)
            nc.sync.dma_start(linw_sb[:], linw_d[:])
            widx_sb = consts.tile((128, N_GATHER), dt.int32)
            nc.sync.dma_start(widx_sb[:], widx_d[:])
            phip_sb = consts.tile((V, N_PAIR * 2 * V), F8)
            nc.sync.dma_start(phip_sb[:], phip_d[:])

            iota_sb = vpack_sb[:, 0:1]
            bias_sb = vpack_sb[:, 1:2]
            negiota_sb = vpack_sb[:, 2:3]

            chars_sb = consts.tile((128, N_GATHER * L), dt.uint8)
            draft_all = consts.tile((F_TOT, N_BLOCK * L), dt.float32)
            out_stage = consts.tile((128, (W // 128) * EMB), dt.float32)
            scratch1 = consts.tile((1, 4), dt.float32)

            # ---- one-time pre-touches absorbing const-DMA lane ticks ----
            # (each DMA has its own HWDGE lane semaphore; an engine that
            # observes a lane's tick once never re-waits on earlier ticks)
            nc.tensor.ldweights(weights=phip_sb[:, 0:1])
            nc.tensor.ldweights(weights=linw_sb[:, 0:1])
            nc.tensor.ldweights(weights=feats[:, 0:1])
            nc.scalar.activation(
                out=scratch1[0:1, 0:1], in_=vpack_sb[0:1, 1:2], func=AF.Copy
            )
            nc.scalar.activation(
                out=scratch1[0:1, 1:2],
                in_=bc_all[0:1, 0:1], func=AF.Copy
            )
            dve_scr = consts.tile((1, 4), dt.float32)
            nc.vector.tensor_copy(dve_scr[0:1, 0:1], bc_all[0:1, 0:1])
            nc.vector.tensor_copy(dve_scr[0:1, 1:2], vpack_sb[0:1, 0:1])
            pool_scratch_reg = nc.gpsimd.alloc_register("poolcar")

            # ---- gather -> stage -> broadcast, software-pipelined ----
            gathers = []
            for i in range(N_GATHER):
                g = nc.gpsimd.indirect_dma_start(
                    out=chars_sb[:, i * L : (i + 1) * L],
                    out_offset=None,
                    in_=ctab_d[:],
                    in_offset=bass.IndirectOffsetOnAxis(
                        ap=widx_sb[:, i : i + 1], axis=0),
                )
                gathers.append(g)

            stages = [None] * N_GATHER

            def emit_stage(i):
                s = nc.sync.dma_start(
                    stage_d[128 * i : 128 * (i + 1), :],
                    chars_sb[:, i * L : (i + 1) * L],
                )
                add_dep_helper(s.ins, gathers[i].ins, reason="stage after gather")
                stages[i] = s

            def emit_bcast(g):
                in_ap = bass_rust.AP(
                    tensor=stage_d[:].tensor,
                    ap=[[0, 127], [CW, GB], [1, SEG]],
                    offset=g * GB * CW,
                )
                bcast = nc.sync.dma_start(
                    bc_all[1:128, g * GB * SEG : (g + 1) * GB * SEG], in_ap
                )
                add_dep_helper(bcast.ins, stages[g].ins, reason="bcast after stage")
                if g + 1 < N_GATHER and stages[g + 1] is not None:
                    add_dep_helper(bcast.ins, stages[g + 1].ins, reason="halo")

            emit_stage(0)
            emit_stage(1)
            emit_bcast(0)
            for i in range(2, N_GATHER):
                emit_stage(i)
                emit_bcast(i - 1)
            emit_bcast(N_GROUP - 1)

            # ---- steady-state pipeline ----
            pm_tiles = {}
            pm_freer = {}          # block -> ins whose completion frees pm slot
            onehots = {}
            # P-path bookkeeping indexed by P-allocation order (pool cycling)
            p_l1 = []              # Pool L1 ins per P-block (relu-tile readers)
            p_fin = []             # DVE finish ins per P-block (tree readers)
            p_tree_jobs = {}       # block -> (tree_t3_ap, l3_ins)
            prev_psl_copy = [None, None]
            group_featsrc = {}

            abs_pool = relup  # reuse pool; (128, SEG) bf16 scratch tiles

            def emit_onehot(c):
                seg = slice(c * SEG, (c + 1) * SEG)
                if OH_ACT[c]:
                    # ACT 2-pass: a = |c - v|; oh = relu(1 - a). Integer
                    # chars make both passes exact: a=0 iff match.
                    a = abs_pool.tile((128, SEG), dt.bfloat16, tag="abs")
                    nc.scalar.activation(
                        out=a[:], in_=bc_all[:, seg], func=AF.Abs,
                        bias=negiota_sb[:], scale=1.0,
                    )
                    onehots[c] = nc.scalar.activation(
                        out=oh_all[:, seg], in_=a[:], func=AF.Relu,
                        bias=1.0, scale=-1.0,
                    )
                else:
                    onehots[c] = nc.vector.tensor_scalar(
                        out=oh_all[:, seg], in0=bc_all[:, seg],
                        scalar1=iota_sb[:], scalar2=None, op0=ALU.is_equal,
                    )

            def emit_taps(b):
                pm = psm.tile((128, 2 * CW), dt.float32, tag="pm")
                pm_tiles[b] = pm
                freer = pm_freer.pop(b - 3, None)
                if freer is not None:
                    ldw = nc.tensor.ldweights(weights=phip_sb[:, 1:2])
                    add_dep_helper(ldw.ins, freer.ins, reason="pm WAR absorb")
                for h in range(2):
                    c = 2 * b + h
                    oh = onehots.pop(c)
                    base = c * SEG
                    for j in range(N_PAIR):
                        lhsT = phip_sb[:, j * 2 * V : (j + 1) * 2 * V].rearrange(
                            "p (two m) -> p two m", two=2
                        )
                        rhs = (
                            oh_all[:, base + j : base + j + CW]
                            .unsqueeze(1)
                            .broadcast_to((128, 2, CW))
                        )
                        mm = nc.tensor.matmul(
                            pm[:, h * CW : (h + 1) * CW],
                            lhsT=lhsT,
                            rhs=rhs,
                            start=(j == 0),
                            stop=(j == N_PAIR - 1),
                            perf_mode=PM.DoubleRow,
                        )
                        if h == 0 and j == 0:
                            add_dep_helper(mm.ins, oh.ins, reason="onehot ready")

            def emit_reduce_D(b):
                pm = pm_tiles.pop(b)
                pm3 = pm[0:F_TOT, :].rearrange("p (w l) -> p w l", l=L)
                r = nc.vector.tensor_reduce(
                    out=draft_all[:, b * L : (b + 1) * L],
                    in_=pm3, axis=mybir.AxisListType.X, op=ALU.max,
                )
                pm_freer[b] = r

            def emit_reduce_P(b):
                pm = pm_tiles.pop(b)
                k = len(p_l1)
                # ACT: relu(y + bias) -> bf16 SBUF; carrier absorbs the Pool
                # tick that frees this relu-pool slot (L1 three P-blocks ago)
                relu = relup.tile((F_TOT, WPB * L), dt.bfloat16, tag="relu")
                if k >= 3:
                    acar = nc.scalar.activation(
                        out=scratch1[0:1, 1:2], in_=scratch1[0:1, 0:1],
                        func=AF.Copy,
                    )
                    add_dep_helper(acar.ins, p_l1[k - 3].ins, reason="relu WAR")
                ev = nc.scalar.activation(
                    out=relu[:], in_=pm[0:F_TOT, :], func=AF.Relu,
                    bias=bias_sb[0:F_TOT, :], scale=1.0 / PHI_SCALE,
                )
                pm_freer[b] = ev
                r3 = relu[:].rearrange("p (w l) -> p w l", l=L)
                t = treep.tile((F_TOT, WPB * 28), dt.bfloat16, tag="tree")
                t1 = t[:, 0 : WPB * 16].rearrange("p (w l) -> p w l", l=16)
                t2 = t[:, WPB * 16 : WPB * 24].rearrange("p (w l) -> p w l", l=8)
                t3 = t[:, WPB * 24 : WPB * 28].rearrange("p (w l) -> p w l", l=4)
                # Pool carrier: tree-pool slot WAR on the DVE finish 3 ago
                # (reg_mov: RegisterMove is walrus-legal on Pool, InstISA
                # engine_nop is not)
                if k >= 3:
                    ncar = nc.gpsimd.reg_mov(pool_scratch_reg, 0)
                    add_dep_helper(ncar.ins, p_fin[k - 3].ins, reason="tree WAR")
                l1 = nc.gpsimd.tensor_tensor(
                    out=t1, in0=r3[:, :, 0:16], in1=r3[:, :, 16:32], op=ALU.max)
                add_dep_helper(l1.ins, ev.ins, reason="tree after evac")
                l2 = nc.gpsimd.tensor_tensor(
                    out=t2, in0=t1[:, :, 0:8], in1=t1[:, :, 8:16], op=ALU.max)
                l3 = nc.gpsimd.tensor_tensor(
                    out=t3, in0=t2[:, :, 0:4], in1=t2[:, :, 4:8], op=ALU.max)
                p_l1.append(l1)
                p_tree_jobs[b] = (t3, l3)

            def emit_fin_P(b):
                t3, l3 = p_tree_jobs.pop(b)
                fin = nc.vector.tensor_reduce(
                    out=feats[0:F_TOT, b * L : (b + 1) * L],
                    in_=t3, axis=mybir.AxisListType.X, op=ALU.max,
                )
                add_dep_helper(fin.ins, l3.ins, reason="finish after tree")
                p_fin.append(fin)
                group_featsrc[b // 4] = fin

            def emit_linear(g):
                gs = slice(g * 4 * L, (g + 1) * 4 * L)
                if PATTERN[g] == "D":
                    # featsr = relu(draft/PHI_SCALE + bias), to bf16, on ACT
                    fr = nc.scalar.activation(
                        out=feats[0:F_TOT, gs], in_=draft_all[:, gs],
                        func=AF.Relu, bias=bias_sb[0:F_TOT, :],
                        scale=1.0 / PHI_SCALE,
                    )
                    group_featsrc[g] = fr
                pl = psl.tile((128, EMB), dt.float32, tag="pl")
                # PE carrier: psl slot WAR on the ACT copy two groups ago
                old = prev_psl_copy.pop(0)
                if old is not None:
                    ldw = nc.tensor.ldweights(weights=phip_sb[:, 2:3])
                    add_dep_helper(ldw.ins, old.ins, reason="psl WAR absorb")
                lm = nc.tensor.matmul(
                    pl[:],
                    lhsT=feats[:, gs],
                    rhs=linw_sb[:],
                    start=True,
                    stop=True,
                )
                add_dep_helper(lm.ins, group_featsrc[g].ins, reason="feats ready")
                cp = nc.scalar.activation(
                    out=out_stage[:, g * EMB : (g + 1) * EMB], in_=pl[:],
                    func=AF.Copy,
                )
                prev_psl_copy.append(cp)

            for it in range(N_BLOCK + FIN_LAG + 2):
                if it < N_BLOCK:
                    emit_onehot(2 * it)
                    emit_onehot(2 * it + 1)
                b = it - 1
                if 0 <= b < N_BLOCK:
                    emit_taps(b)
                    if PATTERN[b // 4] == "D":
                        emit_reduce_D(b)
                    else:
                        emit_reduce_P(b)
                bf = it - FIN_LAG
                if 0 <= bf < N_BLOCK and PATTERN[bf // 4] == "P":
                    emit_fin_P(bf)
                bl = it - FIN_LAG - 1
                if bl >= 3 and (bl + 1) % 4 == 0:
                    emit_linear(bl // 4)

            nc.sync.dma_start(
                out_d[:].rearrange("(g p) e -> p g e", p=128),
                out_stage[:].rearrange("p (g e) -> p g e", e=EMB),
            )

    _split_excess_waits(nc)
    _PROGRAM_CACHE["nc"] = nc
    return nc


def _host_prep(inputs):
    word_idxs = np.asarray(inputs["word_idxs"])
    char_table = np.asarray(inputs["char_table"], dtype=np.int64)
    char_emb = np.asarray(inputs["char_emb"], dtype=np.float32)
    w1 = np.asarray(inputs["w1"], dtype=np.float32)
    w3 = np.asarray(inputs["w3"], dtype=np.float32)
    w5 = np.asarray(inputs["w5"], dtype=np.float32)
    lin_w = np.asarray(inputs["lin_w"], dtype=np.float32)
    lin_b = np.asarray(inputs["lin_b"], dtype=np.float32)
    b1 = np.asarray(inputs["b1"], dtype=np.float32)
    b3 = np.asarray(inputs["b3"], dtype=np.float32)
    b5 = np.asarray(inputs["b5"], dtype=np.float32)

    ctab8 = np.ascontiguousarray(char_table.astype(np.uint8))

    phi = np.zeros((5, V, V), dtype=np.float32)
    phi[0, :, 0:30] = char_emb @ w1[:, :, 0].T
    for j in range(3):
        phi[j, :, 30:70] = char_emb @ w3[:, :, j].T
    for j in range(5):
        phi[j, :, 70:120] = char_emb @ w5[:, :, j].T
    phi *= PHI_SCALE
    # invalid-tail masks ride one-hot row 0 (char 0 never occurs):
    # tap2 covers l in {30,31} for branch3+5, tap4 covers {28,29} for branch5
    # (mask stays raw -224: vs |scaled garbage| < ~96 it still dominates)
    phi[2, 0, 30:120] = MASK_VAL
    phi[4, 0, 70:120] = MASK_VAL

    phi_hi = phi.astype(NP_F8)
    phi_lo = (phi - phi_hi.astype(np.float32)).astype(NP_F8)
    phipack = np.zeros((V, N_PAIR * 2 * V), dtype=NP_F8)
    for j in range(5):
        phipack[:, j * 2 * V : j * 2 * V + V] = phi_hi[j]
        phipack[:, j * 2 * V + V : (j + 1) * 2 * V] = phi_lo[j]

    vpack = np.zeros((128, 3), dtype=np.float32)
    vpack[:, 0] = np.arange(V, dtype=np.float32)
    vpack[0:F_TOT, 1] = np.concatenate([b1, b3, b5])
    vpack[:, 2] = -np.arange(V, dtype=np.float32)

    linw = np.zeros((F_TOT + 1, EMB), dtype=ml_dtypes.bfloat16)
    linw[0:F_TOT] = lin_w.T.astype(ml_dtypes.bfloat16)
    linw[F_TOT] = lin_b.astype(ml_dtypes.bfloat16)

    # bc row 0 pattern: 0 at word-start columns (matches iota[0]=0 so
    # is_equal emits 1s there), 255 (never a char, never matches) elsewhere
    row0 = np.full((1, N_CHUNK * SEG), 255, dtype=np.uint8)
    for seg in range(N_CHUNK):
        for m in range(WPC + 1):
            for k in (0, 1):
                p = m * L + k
                if p < SEG:
                    row0[0, seg * SEG + p] = 0

    ones = np.ones((1, N_BLOCK * L), dtype=ml_dtypes.bfloat16)

    flat = np.ascontiguousarray(word_idxs, dtype=np.int32).reshape(-1)
    common = {
        "ctab8": ctab8,
        "phipack": phipack,
        "vpack": vpack,
        "linw": linw,
        "row0pat": row0,
        "onesrow": ones,
    }
    in_maps = []
    for i in range(N_CORES):
        shard = flat[i * W : (i + 1) * W].reshape(N_GATHER, 128).T
        in_maps.append({"widx": np.ascontiguousarray(shard), **common})
    return in_maps


def run(inputs, trace=False, **kw):
    nc = _build_program()
    in_maps = _host_prep(inputs)
    res = run_bass_kernel_spmd(
        nc, in_maps, core_ids=list(range(N_CORES)), trace=trace, **kw
    )
    out = np.concatenate([r["out"] for r in res.results], axis=0)
    return out.reshape(B, S, EMB).astype(np.float32), res


def kernel(**inputs):
    out, _ = run(inputs, trace=False)
    return out
